# revision 15
# baseline (speedup 1.0000x reference)
"""MoE routing kernel for 8 Trainium2 NeuronCores.

Problem: x, absolute_feature [16, 128, 64, 64] f32; w_gate [8, 65536, 16] f32.
  xs = x.reshape(B, 8, 65536); logits = softmax(einsum('bmi,min->bmn', xs, w_gate))
  top-4 -> renormalized gates -> gather selected channels of x/absolute_feature,
  scale by gates; plus scalar load-balance loss.

Sharding: one MoE block m per core (8 blocks, 8 cores). Each core reads only its
16-channel slice of x/abs and its w_gate[m] slab (~12 MB/core, each input byte
read once across the fleet). The only cross-core reduction is the scalar loss,
done on host from tiny per-core gate/index outputs.

Layout trick: SBUF partition p holds contraction features i in [512p, 512p+512).
w_gate[m] flat-reshaped [128, 8192] lands exactly there, and x rearranged
"(b c)(h j) -> (c h) b j" gives matching [128, 16b, 512j] tiles. The 65536-long
contraction runs as 64 slab matmuls: stationary = x-slab [128, (8j,16b)],
moving = wg-slab [128, (8j,16n)], PSUM [128,128] accumulated; the 8 diagonal
16x16 blocks hold partial logits. They are extracted with 32-aligned block
copies + a block-diagonal mask + one tiny matmul against a tiled-identity
constant that simultaneously sums the halves and duplicates rows to 32
partitions (b twice) for the gather stage.

Top-4 via the DVE max/max_index instruction, softmax via ACT exp with fused
accumulation. Both tensors' selected channels (2 MB) are fetched by a single
indirect DMA from the concatenated [512, 4096] input using on-chip computed
row indices (row = t*256 + b*16 + expert, t = which tensor), scaled by gates
(columns split across DVE and ACT), and streamed out via both HWDGE issuers.
"""

import sys

import numpy as np

try:
    import concourse.bass as bass
except ImportError:  # fall back to the repo checkout location
    for p in ("/opt/trn_rl_repo", "/root/.axon_site/_ro/trn_rl_repo"):
        if p not in sys.path:
            sys.path.insert(0, p)
    import concourse.bass as bass

import concourse.bacc as bacc
import concourse.mybir as mybir
import concourse.tile as tile
from concourse.bass_utils import run_bass_kernel_spmd

B = 16          # batch
M = 8           # moe blocks (= cores)
E = 16          # experts per block (= channels per block)
K = 4           # top-k
HW = 4096       # 64*64
IP = 512        # contraction features per partition (65536 / 128)
NCX = 4         # x DMA chunks along j (issued on sync-HWDGE)
JCX = IP // NCX
NCW = 4         # wg DMA chunks along j (issued on scalar-HWDGE)
JCW = IP // NCW
SLAB = 8        # j's contracted per slab matmul

F32 = mybir.dt.float32
U32 = mybir.dt.uint32

_CACHE = {}


def _build():
    nc = bacc.Bacc(
        "TRN2",
        target_bir_lowering=False,
        debug=False,
        enable_asserts=False,
        num_devices=M,
    )
    # rows 0..255: x slice (b,c); rows 256..511: absolute_feature slice
    xa = nc.dram_tensor("xa", [2 * B * E, HW], F32, kind="ExternalInput").ap()
    wg = nc.dram_tensor("wg", [128, IP * E], F32, kind="ExternalInput").ap()

    oabs = nc.dram_tensor("oabs", [K * B, HW], F32, kind="ExternalOutput").ap()
    oadp = nc.dram_tensor("oadp", [K * B, HW], F32, kind="ExternalOutput").ap()
    og = nc.dram_tensor("og", [B, K], F32, kind="ExternalOutput").ap()
    oidx = nc.dram_tensor("oidx", [B, K], U32, kind="ExternalOutput").ap()

    # constants for diagonal extraction / row duplication (see below)
    msk = nc.inline_tensor(
        np.kron(np.eye(2, dtype=np.float32), np.ones((E, B), np.float32)), "msk"
    ).ap()
    ep = nc.inline_tensor(np.tile(np.eye(E, dtype=np.float32), (2, 1)), "ep").ap()
    e32 = nc.inline_tensor(np.tile(np.eye(B, dtype=np.float32), (1, 2)), "e32").ap()

    # x rearranged so partition p <- features [512p, 512p+512):
    # row (b*16+c), col (h*512+j)  ->  [(c h)=128, b=16, j=512]
    xs_r = xa[0:B * E, :].rearrange("(b c) (h j) -> (c h) b j", b=B, c=E, h=128 // E, j=IP)

    with tile.TileContext(nc) as tc:
        with (
            tc.tile_pool(name="mm", bufs=4) as mm_pool,
            tc.tile_pool(name="gather", bufs=1) as g_pool,
            tc.tile_pool(name="small", bufs=1) as s_pool,
            tc.tile_pool(name="psum", bufs=1, space="PSUM") as p_pool,
        ):
            msks = s_pool.tile([2 * E, 2 * B], F32)
            nc.sync.dma_start(out=msks[:], in_=msk[:])
            eps = s_pool.tile([2 * E, E], F32)
            nc.sync.dma_start(out=eps[:], in_=ep[:])
            e32s = s_pool.tile([B, 2 * B], F32)
            nc.sync.dma_start(out=e32s[:], in_=e32[:])
            biota32 = s_pool.tile([2 * B, 1], F32)
            nc.gpsimd.iota(
                biota32[:], pattern=[[0, 1]], base=0, channel_multiplier=E,
                allow_small_or_imprecise_dtypes=True,
            )

            # descriptor generation spread over three issuers so DMA issue
            # doesn't serialize: x chunk 0 on sync-HWDGE, chunk 1 on
            # gpsimd-SWDGE, wg chunks on scalar-HWDGE
            xt = []
            for q in range(NCX):
                xq = mm_pool.tile([128, B, JCX], F32, tag="xt")
                nc.sync.dma_start(out=xq[:], in_=xs_r[:, :, q * JCX:(q + 1) * JCX])
                xt.append(xq)
            wt = []
            for q in range(NCW):
                wq = mm_pool.tile([128, JCW * E], F32, tag="wt")
                nc.sync.dma_start(out=wq[:], in_=wg[:, q * JCW * E:(q + 1) * JCW * E])
                wt.append(wq)

            # 64 slab matmuls: stationary = wg slab (contiguous), moving = x
            # slab. PSUM P'[(jj,n),(jj',b)]; diag 16x16 blocks accumulate
            # logits^T.
            ps = p_pool.tile([128, SLAB * B], F32)
            nslab = IP // SLAB
            for s in range(nslab):
                j0 = s * SLAB
                qx, jx = divmod(j0, JCX)
                qw, jw = divmod(j0, JCW)
                nc.tensor.matmul(
                    out=ps[:],
                    lhsT=wt[qw][:, jw * E:(jw + SLAB) * E],
                    rhs=xt[qx][:, :, jx:jx + SLAB].rearrange("p b j -> p j b"),
                    start=(s == 0),
                    stop=(s == nslab - 1),
                )

            # diag-block sum via 32-aligned block copies + mask + fold:
            # L[(u,n), b] = sum over slabs jj = u mod 2 of logits^T partials
            dd = s_pool.tile([2 * E, 4 * 2 * B], F32)
            for r in range(4):
                nc.vector.tensor_copy(
                    dd[:, 32 * r:32 * r + 32], ps[32 * r:32 * r + 32, 32 * r:32 * r + 32])
            t1 = s_pool.tile([2 * E, 2 * 2 * B], F32)
            nc.vector.tensor_add(out=t1[:], in0=dd[:, 0:64], in1=dd[:, 64:128])
            s2 = s_pool.tile([2 * E, 2 * B], F32)
            nc.vector.tensor_add(out=s2[:], in0=t1[:, 0:32], in1=t1[:, 32:64])
            nc.vector.tensor_mul(out=s2[:], in0=s2[:], in1=msks[:])
            lh = s_pool.tile([2 * E, B], F32)
            nc.vector.tensor_add(out=lh[:], in0=s2[:, 0:B], in1=s2[:, B:2 * B])
            # sum u-halves and transpose: lg[b, n] = sum_u L[(u,n), b]
            lgp = p_pool.tile([B, E], F32)
            nc.tensor.matmul(out=lgp[:], lhsT=lh[:], rhs=eps[:], start=True, stop=True)
            lgs = s_pool.tile([B, E], F32)
            nc.vector.tensor_copy(lgs[:], lgp[:])
            # duplicate rows to 32 partitions (b twice) for the gather stage
            lg32p = p_pool.tile([2 * B, E], F32)
            nc.tensor.matmul(out=lg32p[:], lhsT=e32s[:], rhs=lgs[:], start=True, stop=True)
            lg = s_pool.tile([2 * B, E], F32)
            nc.vector.tensor_copy(lg[:], lg32p[:])

            # top-8 (sorted desc) values + indices per row; we use the top-4
            mx = s_pool.tile([2 * B, 8], F32)
            idx8 = s_pool.tile([2 * B, 8], U32)
            nc.vector.max(out=mx[:], in_=lg[:])
            nc.vector.max_index(out=idx8[:], in_max=mx[:], in_values=lg[:])

            negmax = s_pool.tile([2 * B, 1], F32)
            nc.vector.tensor_scalar_mul(negmax[:], mx[:, 0:1], -1.0)

            # softmax pieces: Z = sum_n exp(l - max); t4 = sum_top4 exp(l - max)
            et = s_pool.tile([2 * B, E], F32)
            esum = s_pool.tile([2 * B, 1], F32)
            nc.scalar.activation(
                out=et[:], in_=lg[:],
                func=mybir.ActivationFunctionType.Exp,
                bias=negmax[:, 0:1], scale=1.0, accum_out=esum[:],
            )
            e4 = s_pool.tile([2 * B, K], F32)
            t4 = s_pool.tile([2 * B, 1], F32)
            nc.scalar.activation(
                out=e4[:], in_=mx[:, 0:K],
                func=mybir.ActivationFunctionType.Exp,
                bias=negmax[:, 0:1], scale=1.0, accum_out=t4[:],
            )
            # gates = (e4/Z) / (t4/Z + 1e-6) = e4 / (t4 + 1e-6 * Z)
            dn = s_pool.tile([2 * B, 1], F32)
            nc.vector.tensor_scalar_mul(dn[:], esum[:], 1e-6)
            nc.vector.tensor_add(out=dn[:], in0=dn[:], in1=t4[:])
            rc = s_pool.tile([2 * B, 1], F32)
            nc.vector.reciprocal(rc[:], dn[:])
            g4 = s_pool.tile([2 * B, K], F32)
            nc.vector.tensor_scalar_mul(g4[:], e4[:], rc[:, 0:1])

            # gather row index = t*256 + b*16 + idx = 16*(16t+b) + idx
            idx8f = s_pool.tile([2 * B, 8], F32)
            nc.vector.tensor_copy(idx8f[:], idx8[:])
            idxp = s_pool.tile([2 * B, 8], F32)
            nc.vector.tensor_scalar_add(idxp[:], idx8f[:], biota32[:, 0:1])

            gidx = s_pool.tile([128, 1], U32)
            ggate = s_pool.tile([128, 1], F32)
            for kk in range(K):
                nc.vector.tensor_copy(gidx[32 * kk:32 * (kk + 1), 0:1], idxp[:, kk:kk + 1])
                nc.vector.tensor_copy(ggate[32 * kk:32 * (kk + 1), 0:1], g4[:, kk:kk + 1])

            # one indirect DMA gathers all selected 16KB channel rows (2 MB)
            graw = g_pool.tile([128, HW], F32, tag="graw")
            nc.gpsimd.indirect_dma_start(
                out=graw[:], out_offset=None,
                in_=xa[:],
                in_offset=bass.IndirectOffsetOnAxis(ap=gidx[:, 0:1], axis=0),
            )

            # scale by gates; columns split across DVE and ACT
            wsc = g_pool.tile([128, HW], F32, tag="wsc")
            nc.vector.tensor_scalar_mul(wsc[:, 0:HW // 2], graw[:, 0:HW // 2], ggate[:, 0:1])
            nc.scalar.activation(
                out=wsc[:, HW // 2:HW], in_=graw[:, HW // 2:HW],
                func=mybir.ActivationFunctionType.Copy,
                bias=0.0, scale=ggate[:, 0:1],
            )

            # outputs: t=0 half -> weighted x (oadp), t=1 half -> weighted abs
            for kk in range(K):
                nc.scalar.dma_start(
                    out=oadp[kk * B:(kk + 1) * B, :],
                    in_=wsc[32 * kk:32 * kk + B, :])
                nc.sync.dma_start(
                    out=oabs[kk * B:(kk + 1) * B, :],
                    in_=wsc[32 * kk + B:32 * (kk + 1), :])
            nc.sync.dma_start(out=og[:], in_=g4[0:B, :])
            nc.sync.dma_start(out=oidx[:], in_=idx8[0:B, 0:K])

    nc.compile()
    return nc


def _get_nc():
    if "nc" not in _CACHE:
        _CACHE["nc"] = _build()
    return _CACHE["nc"]


def make_in_maps(x, absolute_feature, w_gate):
    x = np.asarray(x, dtype=np.float32)
    a = np.asarray(absolute_feature, dtype=np.float32)
    w = np.asarray(w_gate, dtype=np.float32)
    in_maps = []
    for m in range(M):
        xa = np.concatenate([
            np.ascontiguousarray(x[:, E * m:E * m + E]).reshape(B * E, HW),
            np.ascontiguousarray(a[:, E * m:E * m + E]).reshape(B * E, HW),
        ], axis=0)
        in_maps.append({
            "xa": xa,
            "wg": np.ascontiguousarray(w[m]).reshape(128, IP * E),
        })
    return in_maps


def assemble(results):
    """results: list (per core m) of dicts with oabs/oadp [4*16, 4096],
    og [16,4] f32, oidx [16,4] uint."""
    wabs = np.empty((B, M * K, 64, 64), np.float32)
    wadp = np.empty((B, M * K, 64, 64), np.float32)
    gates = np.zeros((B, M, E), np.float32)
    bi = np.arange(B)[:, None]
    for m in range(M):
        o = results[m]["oabs"].reshape(K, B, HW).transpose(1, 0, 2)
        wabs[:, K * m:K * m + K] = o.reshape(B, K, 64, 64)
        o = results[m]["oadp"].reshape(K, B, HW).transpose(1, 0, 2)
        wadp[:, K * m:K * m + K] = o.reshape(B, K, 64, 64)
        g = results[m]["og"]
        idx = results[m]["oidx"].astype(np.int64)
        gates[bi, m, idx] = g

    imp = gates.sum(axis=0)
    load = (gates > 0).sum(axis=0).astype(np.float32)

    def cv2(v):
        v = v.astype(np.float32).ravel()
        return np.var(v, ddof=1) / (np.mean(v) ** 2 + np.float32(1e-10))

    loss = np.float32((cv2(imp) + cv2(load)) * 0.01)
    return loss, wabs, wadp


def kernel(x, absolute_feature, w_gate):
    nc = _get_nc()
    in_maps = make_in_maps(x, absolute_feature, w_gate)
    res = run_bass_kernel_spmd(nc, in_maps, core_ids=list(range(M)))
    return assemble(res.results)


# revision 16
# speedup vs baseline: 1.1441x; 1.1441x over previous
"""MoE routing kernel for 8 Trainium2 NeuronCores.

Problem: x, absolute_feature [16, 128, 64, 64] f32; w_gate [8, 65536, 16] f32.
  xs = x.reshape(B, 8, 65536); logits = softmax(einsum('bmi,min->bmn', xs, w_gate))
  top-4 -> renormalized gates -> gather selected channels of x/absolute_feature,
  scale by gates; plus scalar load-balance loss.

Sharding: one MoE block m per core (8 blocks, 8 cores). Each core reads only its
16-channel slice of x/abs and its w_gate[m] slab (~12 MB/core, each input byte
read once across the fleet). The only cross-core reduction is the scalar loss,
done on host from tiny per-core gate/index outputs.

Layout trick: SBUF partition p holds contraction features i in [512p, 512p+512).
w_gate[m] flat-reshaped [128, 8192] lands exactly there, and x rearranged
"(b c)(h j) -> (c h) b j" gives matching [128, 16b, 512j] tiles. The 65536-long
contraction runs as 64 slab matmuls: stationary = x-slab [128, (8j,16b)],
moving = wg-slab [128, (8j,16n)], PSUM [128,128] accumulated; the 8 diagonal
16x16 blocks hold partial logits. They are extracted with 32-aligned block
copies + a block-diagonal mask + one tiny matmul against a tiled-identity
constant that simultaneously sums the halves and duplicates rows to 32
partitions (b twice) for the gather stage.

Top-4 via the DVE max/max_index instruction, softmax via ACT exp with fused
accumulation. Both tensors' selected channels (2 MB) are fetched by a single
indirect DMA from the concatenated [512, 4096] input using on-chip computed
row indices (row = t*256 + b*16 + expert, t = which tensor), scaled by gates
(columns split across DVE and ACT), and streamed out via both HWDGE issuers.
"""

import sys

import numpy as np

try:
    import concourse.bass as bass
except ImportError:  # fall back to the repo checkout location
    for p in ("/opt/trn_rl_repo", "/root/.axon_site/_ro/trn_rl_repo"):
        if p not in sys.path:
            sys.path.insert(0, p)
    import concourse.bass as bass

import concourse.bacc as bacc
import concourse.mybir as mybir
import concourse.tile as tile
from concourse.bass_utils import run_bass_kernel_spmd

B = 16          # batch
M = 8           # moe blocks (= cores)
E = 16          # experts per block (= channels per block)
K = 4           # top-k
HW = 4096       # 64*64
IP = 512        # contraction features per partition (65536 / 128)
NCX = 4         # x DMA chunks along j (issued on sync-HWDGE)
JCX = IP // NCX
NCW = 4         # wg DMA chunks along j (issued on scalar-HWDGE)
JCW = IP // NCW
SLAB = 8        # j's contracted per slab matmul

F32 = mybir.dt.float32
U32 = mybir.dt.uint32

_CACHE = {}


def _build():
    nc = bacc.Bacc(
        "TRN2",
        target_bir_lowering=False,
        debug=False,
        enable_asserts=False,
        num_devices=M,
    )
    # rows 0..255: x slice (b,c); rows 256..511: absolute_feature slice
    xa = nc.dram_tensor("xa", [2 * B * E, HW], F32, kind="ExternalInput").ap()
    wg = nc.dram_tensor("wg", [128, IP * E], F32, kind="ExternalInput").ap()

    oabs = nc.dram_tensor("oabs", [K * B, HW], F32, kind="ExternalOutput").ap()
    oadp = nc.dram_tensor("oadp", [K * B, HW], F32, kind="ExternalOutput").ap()
    og = nc.dram_tensor("og", [B, K], F32, kind="ExternalOutput").ap()
    oidx = nc.dram_tensor("oidx", [B, K], U32, kind="ExternalOutput").ap()

    # constants for diagonal extraction / row duplication (see below)
    msk = nc.inline_tensor(
        np.kron(np.eye(2, dtype=np.float32), np.ones((E, B), np.float32)), "msk"
    ).ap()
    ep = nc.inline_tensor(np.tile(np.eye(E, dtype=np.float32), (2, 1)), "ep").ap()
    e32 = nc.inline_tensor(np.tile(np.eye(B, dtype=np.float32), (1, 2)), "e32").ap()

    # x rearranged so partition p <- features [512p, 512p+512):
    # row (b*16+c), col (h*512+j)  ->  [(c h)=128, b=16, j=512]
    xs_r = xa[0:B * E, :].rearrange("(b c) (h j) -> (c h) b j", b=B, c=E, h=128 // E, j=IP)

    with tile.TileContext(nc) as tc:
        with (
            tc.tile_pool(name="mm", bufs=4) as mm_pool,
            tc.tile_pool(name="gather", bufs=1) as g_pool,
            tc.tile_pool(name="small", bufs=1) as s_pool,
            tc.tile_pool(name="psum", bufs=1, space="PSUM") as p_pool,
        ):
            msks = s_pool.tile([2 * E, 2 * B], F32)
            nc.sync.dma_start(out=msks[:], in_=msk[:])
            eps = s_pool.tile([2 * E, E], F32)
            nc.sync.dma_start(out=eps[:], in_=ep[:])
            e32s = s_pool.tile([B, 2 * B], F32)
            nc.sync.dma_start(out=e32s[:], in_=e32[:])
            biota32 = s_pool.tile([2 * B, 1], F32)
            nc.gpsimd.iota(
                biota32[:], pattern=[[0, 1]], base=0, channel_multiplier=E,
                allow_small_or_imprecise_dtypes=True,
            )

            # descriptor generation spread over three issuers so DMA issue
            # doesn't serialize: x chunk 0 on sync-HWDGE, chunk 1 on
            # gpsimd-SWDGE, wg chunks on scalar-HWDGE
            xt = []
            wt = []
            for q in range(NCX):
                xq = mm_pool.tile([128, B, JCX], F32, tag="xt")
                nc.sync.dma_start(out=xq[:], in_=xs_r[:, :, q * JCX:(q + 1) * JCX])
                xt.append(xq)
                wq = mm_pool.tile([128, JCX * E], F32, tag="wt")
                nc.sync.dma_start(out=wq[:], in_=wg[:, q * JCX * E:(q + 1) * JCX * E])
                wt.append(wq)

            # 64 slab matmuls: stationary = wg slab (contiguous), moving = x
            # slab. PSUM P'[(jj,n),(jj',b)]; diag 16x16 blocks accumulate
            # logits^T.
            ps = p_pool.tile([128, SLAB * B], F32)
            nslab = IP // SLAB
            for s in range(nslab):
                j0 = s * SLAB
                qx, jx = divmod(j0, JCX)
                qw, jw = divmod(j0, JCX)
                nc.tensor.matmul(
                    out=ps[:],
                    lhsT=wt[qw][:, jw * E:(jw + SLAB) * E],
                    rhs=xt[qx][:, :, jx:jx + SLAB].rearrange("p b j -> p j b"),
                    start=(s == 0),
                    stop=(s == nslab - 1),
                )

            # diag-block sum via 32-aligned block copies + mask + fold:
            # L[(u,n), b] = sum over slabs jj = u mod 2 of logits^T partials
            dd = s_pool.tile([2 * E, 4 * 2 * B], F32)
            for r in range(4):
                nc.vector.tensor_copy(
                    dd[:, 32 * r:32 * r + 32], ps[32 * r:32 * r + 32, 32 * r:32 * r + 32])
            t1 = s_pool.tile([2 * E, 2 * 2 * B], F32)
            nc.vector.tensor_add(out=t1[:], in0=dd[:, 0:64], in1=dd[:, 64:128])
            s2 = s_pool.tile([2 * E, 2 * B], F32)
            nc.vector.tensor_add(out=s2[:], in0=t1[:, 0:32], in1=t1[:, 32:64])
            nc.vector.tensor_mul(out=s2[:], in0=s2[:], in1=msks[:])
            lh = s_pool.tile([2 * E, B], F32)
            nc.vector.tensor_add(out=lh[:], in0=s2[:, 0:B], in1=s2[:, B:2 * B])
            # sum u-halves and transpose: lg[b, n] = sum_u L[(u,n), b]
            lgp = p_pool.tile([B, E], F32)
            nc.tensor.matmul(out=lgp[:], lhsT=lh[:], rhs=eps[:], start=True, stop=True)
            lgs = s_pool.tile([B, E], F32)
            nc.vector.tensor_copy(lgs[:], lgp[:])
            # duplicate rows to 32 partitions (b twice) for the gather stage
            lg32p = p_pool.tile([2 * B, E], F32)
            nc.tensor.matmul(out=lg32p[:], lhsT=e32s[:], rhs=lgs[:], start=True, stop=True)
            lg = s_pool.tile([2 * B, E], F32)
            nc.vector.tensor_copy(lg[:], lg32p[:])

            # top-8 (sorted desc) values + indices per row; we use the top-4
            mx = s_pool.tile([2 * B, 8], F32)
            idx8 = s_pool.tile([2 * B, 8], U32)
            nc.vector.max(out=mx[:], in_=lg[:])
            nc.vector.max_index(out=idx8[:], in_max=mx[:], in_values=lg[:])

            negmax = s_pool.tile([2 * B, 1], F32)
            nc.vector.tensor_scalar_mul(negmax[:], mx[:, 0:1], -1.0)

            # softmax pieces: Z = sum_n exp(l - max); t4 = sum_top4 exp(l - max)
            et = s_pool.tile([2 * B, E], F32)
            esum = s_pool.tile([2 * B, 1], F32)
            nc.scalar.activation(
                out=et[:], in_=lg[:],
                func=mybir.ActivationFunctionType.Exp,
                bias=negmax[:, 0:1], scale=1.0, accum_out=esum[:],
            )
            e4 = s_pool.tile([2 * B, K], F32)
            t4 = s_pool.tile([2 * B, 1], F32)
            nc.scalar.activation(
                out=e4[:], in_=mx[:, 0:K],
                func=mybir.ActivationFunctionType.Exp,
                bias=negmax[:, 0:1], scale=1.0, accum_out=t4[:],
            )
            # gates = (e4/Z) / (t4/Z + 1e-6) = e4 / (t4 + 1e-6 * Z)
            dn = s_pool.tile([2 * B, 1], F32)
            nc.vector.tensor_scalar_mul(dn[:], esum[:], 1e-6)
            nc.vector.tensor_add(out=dn[:], in0=dn[:], in1=t4[:])
            rc = s_pool.tile([2 * B, 1], F32)
            nc.vector.reciprocal(rc[:], dn[:])
            g4 = s_pool.tile([2 * B, K], F32)
            nc.vector.tensor_scalar_mul(g4[:], e4[:], rc[:, 0:1])

            # gather row index = t*256 + b*16 + idx = 16*(16t+b) + idx
            idx8f = s_pool.tile([2 * B, 8], F32)
            nc.vector.tensor_copy(idx8f[:], idx8[:])
            idxp = s_pool.tile([2 * B, 8], F32)
            nc.vector.tensor_scalar_add(idxp[:], idx8f[:], biota32[:, 0:1])

            gidx = s_pool.tile([128, 1], U32)
            ggate = s_pool.tile([128, 1], F32)
            for kk in range(K):
                nc.vector.tensor_copy(gidx[32 * kk:32 * (kk + 1), 0:1], idxp[:, kk:kk + 1])
                nc.vector.tensor_copy(ggate[32 * kk:32 * (kk + 1), 0:1], g4[:, kk:kk + 1])

            # one indirect DMA gathers all selected 16KB channel rows (2 MB)
            graw = g_pool.tile([128, HW], F32, tag="graw")
            nc.gpsimd.indirect_dma_start(
                out=graw[:], out_offset=None,
                in_=xa[:],
                in_offset=bass.IndirectOffsetOnAxis(ap=gidx[:, 0:1], axis=0),
            )

            # scale by gates; columns split across DVE and ACT
            wsc = g_pool.tile([128, HW], F32, tag="wsc")
            nc.vector.tensor_scalar_mul(wsc[:, 0:HW // 2], graw[:, 0:HW // 2], ggate[:, 0:1])
            nc.scalar.activation(
                out=wsc[:, HW // 2:HW], in_=graw[:, HW // 2:HW],
                func=mybir.ActivationFunctionType.Copy,
                bias=0.0, scale=ggate[:, 0:1],
            )

            # outputs: t=0 half -> weighted x (oadp), t=1 half -> weighted abs
            for kk in range(K):
                nc.scalar.dma_start(
                    out=oadp[kk * B:(kk + 1) * B, :],
                    in_=wsc[32 * kk:32 * kk + B, :])
                nc.sync.dma_start(
                    out=oabs[kk * B:(kk + 1) * B, :],
                    in_=wsc[32 * kk + B:32 * (kk + 1), :])
            nc.sync.dma_start(out=og[:], in_=g4[0:B, :])
            nc.sync.dma_start(out=oidx[:], in_=idx8[0:B, 0:K])

    nc.compile()
    return nc


def _get_nc():
    if "nc" not in _CACHE:
        _CACHE["nc"] = _build()
    return _CACHE["nc"]


def make_in_maps(x, absolute_feature, w_gate):
    x = np.asarray(x, dtype=np.float32)
    a = np.asarray(absolute_feature, dtype=np.float32)
    w = np.asarray(w_gate, dtype=np.float32)
    in_maps = []
    for m in range(M):
        xa = np.concatenate([
            np.ascontiguousarray(x[:, E * m:E * m + E]).reshape(B * E, HW),
            np.ascontiguousarray(a[:, E * m:E * m + E]).reshape(B * E, HW),
        ], axis=0)
        in_maps.append({
            "xa": xa,
            "wg": np.ascontiguousarray(w[m]).reshape(128, IP * E),
        })
    return in_maps


def assemble(results):
    """results: list (per core m) of dicts with oabs/oadp [4*16, 4096],
    og [16,4] f32, oidx [16,4] uint."""
    wabs = np.empty((B, M * K, 64, 64), np.float32)
    wadp = np.empty((B, M * K, 64, 64), np.float32)
    gates = np.zeros((B, M, E), np.float32)
    bi = np.arange(B)[:, None]
    for m in range(M):
        o = results[m]["oabs"].reshape(K, B, HW).transpose(1, 0, 2)
        wabs[:, K * m:K * m + K] = o.reshape(B, K, 64, 64)
        o = results[m]["oadp"].reshape(K, B, HW).transpose(1, 0, 2)
        wadp[:, K * m:K * m + K] = o.reshape(B, K, 64, 64)
        g = results[m]["og"]
        idx = results[m]["oidx"].astype(np.int64)
        gates[bi, m, idx] = g

    imp = gates.sum(axis=0)
    load = (gates > 0).sum(axis=0).astype(np.float32)

    def cv2(v):
        v = v.astype(np.float32).ravel()
        return np.var(v, ddof=1) / (np.mean(v) ** 2 + np.float32(1e-10))

    loss = np.float32((cv2(imp) + cv2(load)) * 0.01)
    return loss, wabs, wadp


def kernel(x, absolute_feature, w_gate):
    nc = _get_nc()
    in_maps = make_in_maps(x, absolute_feature, w_gate)
    res = run_bass_kernel_spmd(nc, in_maps, core_ids=list(range(M)))
    return assemble(res.results)


# revision 17
# speedup vs baseline: 1.1539x; 1.0086x over previous
"""MoE routing kernel for 8 Trainium2 NeuronCores.

Problem: x, absolute_feature [16, 128, 64, 64] f32; w_gate [8, 65536, 16] f32.
  xs = x.reshape(B, 8, 65536); logits = softmax(einsum('bmi,min->bmn', xs, w_gate))
  top-4 -> renormalized gates -> gather selected channels of x/absolute_feature,
  scale by gates; plus scalar load-balance loss.

Sharding: one MoE block m per core (8 blocks, 8 cores). Each core reads only its
16-channel slice of x/abs and its w_gate[m] slab (~12 MB/core, each input byte
read once across the fleet). The only cross-core reduction is the scalar loss,
done on host from tiny per-core gate/index outputs.

Layout trick: SBUF partition p holds contraction features i in [512p, 512p+512).
w_gate[m] flat-reshaped [128, 8192] lands exactly there, and x rearranged
"(b c)(h j) -> (c h) b j" gives matching [128, 16b, 512j] tiles. The 65536-long
contraction runs as 64 slab matmuls: stationary = x-slab [128, (8j,16b)],
moving = wg-slab [128, (8j,16n)], PSUM [128,128] accumulated; the 8 diagonal
16x16 blocks hold partial logits. They are extracted with 32-aligned block
copies + a block-diagonal mask + one tiny matmul against a tiled-identity
constant that simultaneously sums the halves and duplicates rows to 32
partitions (b twice) for the gather stage.

Top-4 via the DVE max/max_index instruction, softmax via ACT exp with fused
accumulation. Both tensors' selected channels (2 MB) are fetched by a single
indirect DMA from the concatenated [512, 4096] input using on-chip computed
row indices (row = t*256 + b*16 + expert, t = which tensor), scaled by gates
(columns split across DVE and ACT), and streamed out via both HWDGE issuers.
"""

import sys

import numpy as np

try:
    import concourse.bass as bass
except ImportError:  # fall back to the repo checkout location
    for p in ("/opt/trn_rl_repo", "/root/.axon_site/_ro/trn_rl_repo"):
        if p not in sys.path:
            sys.path.insert(0, p)
    import concourse.bass as bass

import concourse.bacc as bacc
import concourse.mybir as mybir
import concourse.tile as tile
from concourse.bass_utils import run_bass_kernel_spmd

B = 16          # batch
M = 8           # moe blocks (= cores)
E = 16          # experts per block (= channels per block)
K = 4           # top-k
HW = 4096       # 64*64
IP = 512        # contraction features per partition (65536 / 128)
NCX = 4         # x DMA chunks along j (issued on sync-HWDGE)
JCX = IP // NCX
NCW = 4         # wg DMA chunks along j (issued on scalar-HWDGE)
JCW = IP // NCW
SLAB = 8        # j's contracted per slab matmul

F32 = mybir.dt.float32
U32 = mybir.dt.uint32

_CACHE = {}


def _build():
    nc = bacc.Bacc(
        "TRN2",
        target_bir_lowering=False,
        debug=False,
        enable_asserts=False,
        num_devices=M,
    )
    # rows 0..255: x slice (b,c); rows 256..511: absolute_feature slice
    xa = nc.dram_tensor("xa", [2 * B * E, HW], F32, kind="ExternalInput").ap()
    wg = nc.dram_tensor("wg", [128, IP * E], F32, kind="ExternalInput").ap()

    oabs = nc.dram_tensor("oabs", [K * B, HW], F32, kind="ExternalOutput").ap()
    oadp = nc.dram_tensor("oadp", [K * B, HW], F32, kind="ExternalOutput").ap()
    og = nc.dram_tensor("og", [B, K], F32, kind="ExternalOutput").ap()
    oidx = nc.dram_tensor("oidx", [B, K], U32, kind="ExternalOutput").ap()

    # constants for diagonal extraction / row duplication (see below)
    msk = nc.inline_tensor(
        np.kron(np.eye(2, dtype=np.float32), np.ones((E, B), np.float32)), "msk"
    ).ap()
    ep = nc.inline_tensor(np.tile(np.eye(E, dtype=np.float32), (2, 1)), "ep").ap()
    e32 = nc.inline_tensor(np.tile(np.eye(B, dtype=np.float32), (1, 2)), "e32").ap()

    # x rearranged so partition p <- features [512p, 512p+512):
    # row (b*16+c), col (h*512+j)  ->  [(c h)=128, b=16, j=512]
    xs_r = xa[0:B * E, :].rearrange("(b c) (h j) -> (c h) b j", b=B, c=E, h=128 // E, j=IP)

    with tile.TileContext(nc) as tc:
        with (
            tc.tile_pool(name="mm", bufs=4) as mm_pool,
            tc.tile_pool(name="gather", bufs=1) as g_pool,
            tc.tile_pool(name="small", bufs=1) as s_pool,
            tc.tile_pool(name="psum", bufs=1, space="PSUM") as p_pool,
        ):
            msks = s_pool.tile([2 * E, 2 * B], F32)
            nc.sync.dma_start(out=msks[:], in_=msk[:])
            eps = s_pool.tile([2 * E, E], F32)
            nc.sync.dma_start(out=eps[:], in_=ep[:])
            e32s = s_pool.tile([B, 2 * B], F32)
            nc.sync.dma_start(out=e32s[:], in_=e32[:])
            biota32 = s_pool.tile([2 * B, 1], F32)
            nc.gpsimd.iota(
                biota32[:], pattern=[[0, 1]], base=0, channel_multiplier=E,
                allow_small_or_imprecise_dtypes=True,
            )

            # descriptor generation spread over three issuers so DMA issue
            # doesn't serialize: x chunk 0 on sync-HWDGE, chunk 1 on
            # gpsimd-SWDGE, wg chunks on scalar-HWDGE
            xt = []
            wt = []
            for q in range(NCX):
                xq = mm_pool.tile([128, B, JCX], F32, tag="xt")
                nc.sync.dma_start(out=xq[:], in_=xs_r[:, :, q * JCX:(q + 1) * JCX])
                xt.append(xq)
                wq = mm_pool.tile([128, JCX * E], F32, tag="wt")
                nc.sync.dma_start(out=wq[:], in_=wg[:, q * JCX * E:(q + 1) * JCX * E])
                wt.append(wq)

            # 64 slab matmuls: stationary = wg slab (contiguous), moving = x
            # slab. PSUM P'[(jj,n),(jj',b)]; diag 16x16 blocks accumulate
            # logits^T.
            ps = p_pool.tile([128, SLAB * B], F32)
            nslab = IP // SLAB
            for s in range(nslab):
                j0 = s * SLAB
                qx, jx = divmod(j0, JCX)
                qw, jw = divmod(j0, JCX)
                nc.tensor.matmul(
                    out=ps[:],
                    lhsT=wt[qw][:, jw * E:(jw + SLAB) * E],
                    rhs=xt[qx][:, :, jx:jx + SLAB].rearrange("p b j -> p j b"),
                    start=(s == 0),
                    stop=(s == nslab - 1),
                )

            # diag-block sum via 32-aligned block copies + mask + fold:
            # L[(u,n), b] = sum over slabs jj = u mod 2 of logits^T partials
            dd = s_pool.tile([2 * E, 4 * 2 * B], F32)
            for r in range(4):
                nc.vector.tensor_copy(
                    dd[:, 32 * r:32 * r + 32], ps[32 * r:32 * r + 32, 32 * r:32 * r + 32])
            t1 = s_pool.tile([2 * E, 2 * 2 * B], F32)
            nc.vector.tensor_add(out=t1[:], in0=dd[:, 0:64], in1=dd[:, 64:128])
            s2 = s_pool.tile([2 * E, 2 * B], F32)
            nc.vector.tensor_add(out=s2[:], in0=t1[:, 0:32], in1=t1[:, 32:64])
            nc.vector.tensor_mul(out=s2[:], in0=s2[:], in1=msks[:])
            lh = s_pool.tile([2 * E, B], F32)
            nc.vector.tensor_add(out=lh[:], in0=s2[:, 0:B], in1=s2[:, B:2 * B])
            # sum u-halves and transpose: lg[b, n] = sum_u L[(u,n), b]
            lgp = p_pool.tile([B, E], F32)
            nc.tensor.matmul(out=lgp[:], lhsT=lh[:], rhs=eps[:], start=True, stop=True)
            lgs = s_pool.tile([B, E], F32)
            nc.vector.tensor_copy(lgs[:], lgp[:])
            # duplicate rows to 32 partitions (b twice) for the gather stage
            lg32p = p_pool.tile([2 * B, E], F32)
            nc.tensor.matmul(out=lg32p[:], lhsT=e32s[:], rhs=lgs[:], start=True, stop=True)
            lg = s_pool.tile([2 * B, E], F32)
            nc.vector.tensor_copy(lg[:], lg32p[:])

            # top-8 (sorted desc) values + indices per row; we use the top-4
            mx = s_pool.tile([2 * B, 8], F32)
            idx8 = s_pool.tile([2 * B, 8], U32)
            nc.vector.max(out=mx[:], in_=lg[:])
            nc.vector.max_index(out=idx8[:], in_max=mx[:], in_values=lg[:])

            negmax = s_pool.tile([2 * B, 1], F32)
            nc.vector.tensor_scalar_mul(negmax[:], mx[:, 0:1], -1.0)

            # softmax pieces: Z = sum_n exp(l - max); t4 = sum_top4 exp(l - max)
            et = s_pool.tile([2 * B, E], F32)
            esum = s_pool.tile([2 * B, 1], F32)
            nc.scalar.activation(
                out=et[:], in_=lg[:],
                func=mybir.ActivationFunctionType.Exp,
                bias=negmax[:, 0:1], scale=1.0, accum_out=esum[:],
            )
            e4 = s_pool.tile([2 * B, K], F32)
            t4 = s_pool.tile([2 * B, 1], F32)
            nc.scalar.activation(
                out=e4[:], in_=mx[:, 0:K],
                func=mybir.ActivationFunctionType.Exp,
                bias=negmax[:, 0:1], scale=1.0, accum_out=t4[:],
            )
            # gates = (e4/Z) / (t4/Z + 1e-6) = e4 / (t4 + 1e-6 * Z)
            dn = s_pool.tile([2 * B, 1], F32)
            nc.vector.tensor_scalar_mul(dn[:], esum[:], 1e-6)
            nc.vector.tensor_add(out=dn[:], in0=dn[:], in1=t4[:])
            rc = s_pool.tile([2 * B, 1], F32)
            nc.vector.reciprocal(rc[:], dn[:])
            g4 = s_pool.tile([2 * B, K], F32)
            nc.vector.tensor_scalar_mul(g4[:], e4[:], rc[:, 0:1])

            # gather row index = t*256 + b*16 + idx = 16*(16t+b) + idx
            idx8f = s_pool.tile([2 * B, 8], F32)
            nc.vector.tensor_copy(idx8f[:], idx8[:])
            idxp = s_pool.tile([2 * B, 8], F32)
            nc.vector.tensor_scalar_add(idxp[:], idx8f[:], biota32[:, 0:1])

            # kk-half pipeline: gather half A (kk 0,1), then scale/store A
            # while half B (kk 2,3) gathers. Separate index/gate tiles keep
            # the dependencies exact.
            gh = []
            for h in range(2):
                gi = s_pool.tile([64, 1], U32, tag=f"gidx{h}")
                gg = s_pool.tile([64, 1], F32, tag=f"ggate{h}")
                for j in range(2):
                    kk = 2 * h + j
                    nc.vector.tensor_copy(gi[32 * j:32 * (j + 1), 0:1], idxp[:, kk:kk + 1])
                    nc.vector.tensor_copy(gg[32 * j:32 * (j + 1), 0:1], g4[:, kk:kk + 1])
                gh.append((gi, gg))

            graw = g_pool.tile([128, HW], F32, tag="graw")
            wsc = g_pool.tile([128, HW], F32, tag="wsc")
            for h in range(2):
                gi, gg = gh[h]
                p0 = 64 * h
                nc.gpsimd.indirect_dma_start(
                    out=graw[p0:p0 + 64, :], out_offset=None,
                    in_=xa[:],
                    in_offset=bass.IndirectOffsetOnAxis(ap=gi[:, 0:1], axis=0),
                )
                # scale by gates; columns split across DVE and ACT
                nc.vector.tensor_scalar_mul(
                    wsc[p0:p0 + 64, 0:HW // 2], graw[p0:p0 + 64, 0:HW // 2], gg[:, 0:1])
                nc.scalar.activation(
                    out=wsc[p0:p0 + 64, HW // 2:HW], in_=graw[p0:p0 + 64, HW // 2:HW],
                    func=mybir.ActivationFunctionType.Copy,
                    bias=0.0, scale=gg[:, 0:1],
                )
                # t=0 sub-half -> weighted x (oadp), t=1 -> weighted abs
                for j in range(2):
                    kk = 2 * h + j
                    nc.scalar.dma_start(
                        out=oadp[kk * B:(kk + 1) * B, :],
                        in_=wsc[p0 + 32 * j:p0 + 32 * j + B, :])
                    nc.sync.dma_start(
                        out=oabs[kk * B:(kk + 1) * B, :],
                        in_=wsc[p0 + 32 * j + B:p0 + 32 * (j + 1), :])
            nc.sync.dma_start(out=og[:], in_=g4[0:B, :])
            nc.sync.dma_start(out=oidx[:], in_=idx8[0:B, 0:K])

    nc.compile()
    return nc


def _get_nc():
    if "nc" not in _CACHE:
        _CACHE["nc"] = _build()
    return _CACHE["nc"]


def make_in_maps(x, absolute_feature, w_gate):
    x = np.asarray(x, dtype=np.float32)
    a = np.asarray(absolute_feature, dtype=np.float32)
    w = np.asarray(w_gate, dtype=np.float32)
    in_maps = []
    for m in range(M):
        xa = np.concatenate([
            np.ascontiguousarray(x[:, E * m:E * m + E]).reshape(B * E, HW),
            np.ascontiguousarray(a[:, E * m:E * m + E]).reshape(B * E, HW),
        ], axis=0)
        in_maps.append({
            "xa": xa,
            "wg": np.ascontiguousarray(w[m]).reshape(128, IP * E),
        })
    return in_maps


def assemble(results):
    """results: list (per core m) of dicts with oabs/oadp [4*16, 4096],
    og [16,4] f32, oidx [16,4] uint."""
    wabs = np.empty((B, M * K, 64, 64), np.float32)
    wadp = np.empty((B, M * K, 64, 64), np.float32)
    gates = np.zeros((B, M, E), np.float32)
    bi = np.arange(B)[:, None]
    for m in range(M):
        o = results[m]["oabs"].reshape(K, B, HW).transpose(1, 0, 2)
        wabs[:, K * m:K * m + K] = o.reshape(B, K, 64, 64)
        o = results[m]["oadp"].reshape(K, B, HW).transpose(1, 0, 2)
        wadp[:, K * m:K * m + K] = o.reshape(B, K, 64, 64)
        g = results[m]["og"]
        idx = results[m]["oidx"].astype(np.int64)
        gates[bi, m, idx] = g

    imp = gates.sum(axis=0)
    load = (gates > 0).sum(axis=0).astype(np.float32)

    def cv2(v):
        v = v.astype(np.float32).ravel()
        return np.var(v, ddof=1) / (np.mean(v) ** 2 + np.float32(1e-10))

    loss = np.float32((cv2(imp) + cv2(load)) * 0.01)
    return loss, wabs, wadp


def kernel(x, absolute_feature, w_gate):
    nc = _get_nc()
    in_maps = make_in_maps(x, absolute_feature, w_gate)
    res = run_bass_kernel_spmd(nc, in_maps, core_ids=list(range(M)))
    return assemble(res.results)
